# revision 9
# baseline (speedup 1.0000x reference)
"""DiffusionBlock TRN2 kernel: spectral diffusion + sparse COO gradient op +
MLP + residual LayerNorm, sharded over 8 NeuronCores by node rows.

v5: the sparse segment-sum runs as fp8 matmuls between a host-assembled
dense edge-source stream (xe[slot] = x[col_e], streamed at full DMA rate —
no per-edge gather descriptors) and narrow banded one-hot operands (edges
sorted by destination row within each 128-row block, vals baked in).

Self-contained: hardcodes all shapes; builds + compiles a Bass program at
call time (specialized to the edge distribution), runs SPMD on cores 0-7.
"""
import sys
sys.path.insert(0, '/opt/trn_rl_repo')
import numpy as np
import ml_dtypes
import concourse.mybir as mybir
from concourse.bass import Bass
from concourse.tile import TileContext
from concourse import bass_utils

dt = mybir.dt
FP8 = ml_dtypes.float8_e4m3

# problem dims (hardcoded per contract)
N, C, K, G, E = 65536, 256, 128, 32, 2097152
LN_EPS = 1e-5
NCORES = 8
RPC = N // NCORES          # rows per core = 8192
GPC = G // NCORES          # graphs per core = 4
NPG = N // G               # nodes per graph = 2048
NBLK = RPC // 128          # 128-row blocks per core = 64
BPG = NPG // 128           # blocks per graph = 16


# ---------------------------------------------------------------- BIR fixups
_wspill = [0]


def _legalize_waits(nc):
    """This walrus accepts at most 1 sync-wait per instruction (2 for
    EventSemaphore). Spill extras into EventSemaphore insts inserted just
    before, same engine. Also run codegen_inst_isa_subclasses (Bacc does it,
    raw Bass doesn't) so extended-ISA insts get their raw words."""
    mybir.codegen_inst_isa_subclasses(nc)
    f = nc.m.functions[0]
    for bb in f.blocks:
        out = []
        changed = False
        for ins in bb.instructions:
            si = ins.sync_info
            cap = 2 if ins.opcode == 'EventSemaphore' else 1
            if si is not None and si.on_wait is not None and len(si.on_wait) > cap:
                waits = list(si.on_wait)
                keep, spill = waits[:cap], waits[cap:]
                while spill:
                    batch, spill = spill[:2], spill[2:]
                    _wspill[0] += 1
                    es = mybir.InstEventSemaphore(
                        name=f"WSPILL-{_wspill[0]}", ins=[], outs=[])
                    es.engine = ins.engine
                    es.sync_info = mybir.SyncInfo(on_wait=batch, on_update=[])
                    out.append(es)
                si.on_wait = keep
                changed = True
            out.append(ins)
        if changed:
            bb.instructions = out
    return nc


# ---------------------------------------------------------------- host prep
def _prepare(inputs):
    x = np.asarray(inputs["x"], np.float32)
    evals = np.asarray(inputs["evals_batch"], np.float32)
    evecs = np.asarray(inputs["evecs"], np.float32)
    mass = np.asarray(inputs["mass"], np.float32)
    row = np.asarray(inputs["row"]).astype(np.int64)
    col = np.asarray(inputs["col"]).astype(np.int64)
    vals = np.asarray(inputs["vals"], np.float32)
    t_params = np.asarray(inputs["t_params"], np.float32)
    grad_W = np.asarray(inputs["grad_W"], np.float32)
    grad_b = np.asarray(inputs["grad_b"], np.float32)
    W1 = np.asarray(inputs["W1"], np.float32)
    b1 = np.asarray(inputs["b1"], np.float32)
    W2 = np.asarray(inputs["W2"], np.float32)
    b2 = np.asarray(inputs["b2"], np.float32)
    ln_g = np.asarray(inputs["ln_g"], np.float32)
    ln_b = np.asarray(inputs["ln_b"], np.float32)

    xf8_full = x.astype(FP8)
    x16_full = x.astype(np.float16)

    # fold grad_W / grad_b into the second half of W1 (host, fp64 for accuracy)
    W1a = W1[:, :C]
    W1b = W1[:, C:]
    Wfold = (W1b.astype(np.float64) @ grad_W.astype(np.float64)).astype(np.float32)
    b1f_np = b1 + (W1b.astype(np.float64) @ grad_b.astype(np.float64)).astype(np.float32)

    # decay[g,k,c] = exp(-|t_c| * max(ev_gk, 0))
    t = np.abs(t_params)
    ev = np.maximum(evals.reshape(G, K), 0.0)
    decay = np.exp(-ev[:, :, None] * t[None, None, :]).astype(np.float32)  # [G,K,C]

    em_full = (evecs * mass[:, None]).astype(np.float16)   # [N,K]
    ev16_full = evecs.astype(np.float16)

    # ---- edge partitioning: per core, per 128-row dest block, sorted by
    # lrow (dest row within block). Chunks split at lrow 64-boundaries so
    # each banded one-hot is a 64-wide PSUM window at offset {0,64}
    # (PE tile_position: 32-aligned, quadrant 3 unusable on trn2). ----
    BW = 64
    core_of = row >> 13               # row // 8192
    percore = []
    # counts per (core, block, quarter)
    counts_q = np.zeros((NCORES, NBLK, 2), np.int64)
    for i in range(NCORES):
        sel = np.where(core_of == i)[0]
        r = row[sel] - i * RPC
        c_ = col[sel]
        v = vals[sel]
        blk = r >> 7
        lrow = r & 127
        order = np.lexsort((lrow, blk))
        blk_s = blk[order]
        lrow_s = lrow[order]
        percore.append((blk_s, lrow_s, c_[order], v[order]))
        np.add.at(counts_q[i], (blk_s, lrow_s >> 6), 1)

    # uniform chunk counts across cores (SPMD: one program)
    CBq = ((counts_q.max(0) + 127) // 128).astype(np.int64)  # [NBLK, 2]
    CB = CBq.sum(1)                                          # chunks per block
    cum_CB = np.concatenate([[0], np.cumsum(CB)]).astype(np.int64)
    CTsum = int(cum_CB[-1])
    # chunk id -> psum window offset (64 * half)
    cum_CBq = np.concatenate([[0], np.cumsum(CBq.reshape(-1))]).astype(np.int64)
    off_chunk = np.zeros(CTsum, np.int64)
    for b in range(NBLK):
        for q in range(2):
            s = int(cum_CBq[b * 2 + q])
            e = int(cum_CBq[b * 2 + q + 1])
            off_chunk[s:e] = 64 * q

    in_maps = []
    for i in range(NCORES):
        blk_s, lrow_s, c_s, v_s = percore[i]
        half = lrow_s >> 6
        grp = blk_s * 2 + half                   # sorted ascending
        gstart_core = np.concatenate(
            [[0], np.cumsum(np.bincount(grp, minlength=NBLK * 2))])
        pos = np.arange(len(grp)) - gstart_core[grp]
        chunk = pos >> 7
        slot = pos & 127
        cid = cum_CBq[grp] + chunk

        xe = np.zeros((128, CTsum, C), FP8)
        xe[slot, cid, :] = xf8_full[c_s]
        oh0 = np.zeros((128, NBLK, 128), FP8)
        m0 = cid == cum_CB[blk_s]                # first chunk of the block
        oh0[slot[m0], blk_s[m0], lrow_s[m0]] = v_s[m0]
        band = np.zeros((128, CTsum, BW), FP8)
        mb = ~m0
        rel = lrow_s[mb] - off_chunk[cid[mb]]
        assert rel.min() >= 0 and rel.max() < BW
        band[slot[mb], cid[mb], rel] = v_s[mb]

        sl_rows = slice(i * RPC, (i + 1) * RPC)
        sl_g = slice(i * GPC, (i + 1) * GPC)
        evT16 = np.ascontiguousarray(
            ev16_full[sl_rows].reshape(GPC, NPG, K).transpose(0, 2, 1)
        ).reshape(GPC * K, NPG)

        c16 = np.zeros((128, 1664), np.float16)
        off = 0
        W1aT = W1a.T.astype(np.float16)
        WfT = Wfold.T.astype(np.float16)
        for Wt in (W1aT, WfT):
            for k in range(2):
                for m in range(2):
                    c16[:, off:off + 128] = Wt[k * 128:(k + 1) * 128,
                                               m * 128:(m + 1) * 128]
                    off += 128
        W2T = W2.T.astype(np.float16)
        for k in range(2):
            c16[:, off:off + 256] = W2T[k * 128:(k + 1) * 128, :]
            off += 256
        c16[:, off:off + 128] = np.eye(128, dtype=np.float16)
        off += 128
        assert off == 1664

        c32 = np.zeros((128, 516), np.float32)
        c32[:, 0] = b1f_np[:128]
        c32[:, 1] = b1f_np[128:]
        c32[:, 2:258] = np.broadcast_to(ln_g, (128, C))
        c32[:, 258:514] = np.broadcast_to(ln_b, (128, C))
        c32[:, 514] = LN_EPS

        in_maps.append({
            "xe": xe,
            "oh0": oh0,
            "band": band,
            "x16": np.ascontiguousarray(x16_full[sl_rows]),
            "xr": np.ascontiguousarray(x[sl_rows] + b2[None, :]),
            "em16": np.ascontiguousarray(em_full[sl_rows]),
            "evT16": evT16,
            "decay": np.ascontiguousarray(decay[sl_g]),
            "c16": c16,
            "c32": c32,
        })
    plan = dict(CB=CB, CBq=CBq, cum_CB=cum_CB, cum_CBq=cum_CBq,
                CTsum=CTsum, BW=BW, off_chunk=off_chunk)
    return in_maps, plan


# ---------------------------------------------------------------- program
def _build(plan):
    CB, cum_CB = plan["CB"], plan["cum_CB"]
    CBq, cum_CBq = plan["CBq"], plan["cum_CBq"]
    CTsum, BW = plan["CTsum"], plan["BW"]
    off_chunk = plan["off_chunk"]

    nc = Bass()
    xe_h = nc.dram_tensor("xe", [128, CTsum, C], dt.float8e4,
                          kind="ExternalInput")
    oh0_h = nc.dram_tensor("oh0", [128, NBLK, 128], dt.float8e4,
                           kind="ExternalInput")
    band_h = nc.dram_tensor("band", [128, CTsum, BW], dt.float8e4,
                            kind="ExternalInput")
    x16_h = nc.dram_tensor("x16", [RPC, C], dt.float16, kind="ExternalInput")
    xr_h = nc.dram_tensor("xr", [RPC, C], dt.float32, kind="ExternalInput")
    em_h = nc.dram_tensor("em16", [RPC, K], dt.float16, kind="ExternalInput")
    evT_h = nc.dram_tensor("evT16", [GPC * K, NPG], dt.float16, kind="ExternalInput")
    dec_h = nc.dram_tensor("decay", [GPC, K, C], dt.float32, kind="ExternalInput")
    c16_h = nc.dram_tensor("c16", [128, 1664], dt.float16, kind="ExternalInput")
    c32_h = nc.dram_tensor("c32", [128, 516], dt.float32, kind="ExternalInput")
    out_h = nc.dram_tensor("out", [RPC, C], dt.float16, kind="ExternalOutput")

    TS = mybir.AluOpType
    AF = mybir.ActivationFunctionType

    with TileContext(nc) as tc:
        with tc.tile_pool(name="consts", bufs=1) as cp, \
             tc.tile_pool(name="spec", bufs=2) as sp, \
             tc.tile_pool(name="edge", bufs=2) as ep, \
             tc.tile_pool(name="segp", bufs=2) as sg, \
             tc.tile_pool(name="mlp", bufs=2) as mp, \
             tc.tile_pool(name="ln", bufs=3) as lp, \
             tc.tile_pool(name="ps", bufs=1, space="PSUM") as pp:
            c16 = cp.tile([128, 1664], dt.float16)
            c32 = cp.tile([128, 516], dt.float32)
            nc.sync.dma_start(c16[:], c16_h[:, :])
            nc.sync.dma_start(c32[:], c32_h[:, :])
            oh0_t = cp.tile([128, NBLK, 128], dt.float8e4)
            nc.sync.dma_start(oh0_t[:], oh0_h[:, :, :])
            W1aT = [[c16[:, (k * 2 + m) * 128:(k * 2 + m + 1) * 128]
                     for m in range(2)] for k in range(2)]
            WfT = [[c16[:, 512 + (k * 2 + m) * 128:512 + (k * 2 + m + 1) * 128]
                    for m in range(2)] for k in range(2)]
            W2T = [c16[:, 1024 + k * 256:1024 + (k + 1) * 256] for k in range(2)]
            ident = c16[:, 1536:1664]
            b1f = [c32[:, m:m + 1] for m in range(2)]
            grep = c32[:, 2:258]
            brep = c32[:, 258:514]
            eps_ap = c32[:, 514:515]
            zero_ap = c32[:, 515:516]

            def seg_block(B, band_t, bc0, segT_t):
                nch = int(CB[B])
                cid0 = int(cum_CB[B])
                xe_t = ep.tile([128, nch, C], dt.float8e4, tag="xe", bufs=4,
                               name="xe_t")
                nc.sync.dma_start(xe_t[:],
                                  xe_h[:, cid0:cid0 + nch, :])
                psg = pp.tile([128, C], dt.float32, tag="grad", bufs=2,
                              name="psg")
                nc.tensor.matmul(psg[:], oh0_t[:, B, :], xe_t[:, 0, :],
                                 start=True, stop=(nch == 1))
                for j in range(1, nch):
                    o = int(off_chunk[cid0 + j])
                    nc.tensor.matmul(psg[o:o + BW, :],
                                     band_t[:, cid0 - bc0 + j, :],
                                     xe_t[:, j, :],
                                     start=False, stop=(j == nch - 1))
                segNM = sg.tile([128, C], dt.float16, tag="segNM", name="segNM")
                nc.scalar.copy(segNM[:], psg[:])
                for h2 in range(2):
                    tp = pp.tile([128, 128], dt.float16, tag="tp", bufs=1,
                                 name="tp")
                    nc.tensor.transpose(tp[:], segNM[:, h2 * 128:(h2 + 1) * 128],
                                        ident)
                    nc.vector.tensor_copy(
                        segT_t[:, h2, (B % 4) * 128:(B % 4 + 1) * 128], tp[:])

            def mlp_group(B, segT_t, diffT):
                n0 = (B - 3) * 128
                goff = n0 % NPG
                gslice = slice(goff, goff + 512)
                hT = mp.tile([128, 2, 512], dt.float16, tag="hT", name="hT")
                for m in range(2):
                    ph = pp.tile([128, 512], dt.float32, tag="h", bufs=2,
                                 name="ph")
                    nc.tensor.matmul(ph[:], W1aT[0][m], diffT[0][:, gslice],
                                     start=True, stop=False)
                    nc.tensor.matmul(ph[:], W1aT[1][m], diffT[1][:, gslice],
                                     start=False, stop=False)
                    for k in range(2):
                        nc.tensor.matmul(ph[:], WfT[k][m], segT_t[:, k, :],
                                         start=False, stop=(k == 1))
                    nc.scalar.activation(hT[:, m, :], ph[:], AF.Relu,
                                         bias=b1f[m], scale=1.0)
                for tt in range(4):
                    nt = B - 3 + tt
                    py = pp.tile([128, C], dt.float32, tag="y", bufs=1,
                                 name="py")
                    for k in range(2):
                        nc.tensor.matmul(py[:], hT[:, k, tt * 128:(tt + 1) * 128],
                                         W2T[k], start=(k == 0), stop=(k == 1))
                    xrt = lp.tile([128, C], dt.float32, tag="xr", name="xrt")
                    nc.sync.dma_start(xrt[:], xr_h[nt * 128:(nt + 1) * 128, :])
                    y = lp.tile([128, C], dt.float32, tag="y", name="y")
                    nc.vector.tensor_add(y[:], py[:], xrt[:])
                    nsum = lp.tile([128, 1], dt.float32, tag="s0", name="s0")
                    nc.vector.tensor_reduce(nsum[:], y[:], mybir.AxisListType.X,
                                            TS.add, negate=True)
                    nmu = lp.tile([128, 1], dt.float32, tag="s1", name="s1")
                    nc.vector.tensor_scalar_mul(nmu[:], nsum[:], 1.0 / C)
                    sq = lp.tile([128, C], dt.float32, tag="sq", name="sq")
                    sqs = lp.tile([128, 1], dt.float32, tag="s2", name="s2")
                    nc.scalar.activation(sq[:], y[:], AF.Square,
                                         bias=zero_ap, accum_out=sqs[:])
                    ex2 = lp.tile([128, 1], dt.float32, tag="s3", name="s3")
                    nc.vector.tensor_scalar_mul(ex2[:], sqs[:], 1.0 / C)
                    mu2 = lp.tile([128, 1], dt.float32, tag="s4", name="s4")
                    nc.vector.tensor_mul(mu2[:], nmu[:], nmu[:])
                    var = lp.tile([128, 1], dt.float32, tag="s5", name="s5")
                    nc.vector.tensor_sub(var[:], ex2[:], mu2[:])
                    sd = lp.tile([128, 1], dt.float32, tag="s6", name="s6")
                    nc.scalar.activation(sd[:], var[:], AF.Sqrt, bias=eps_ap)
                    rstd = lp.tile([128, 1], dt.float32, tag="s7", name="s7")
                    nc.vector.reciprocal(rstd[:], sd[:])
                    yn = lp.tile([128, C], dt.float32, tag="yn", name="yn")
                    nc.vector.scalar_tensor_tensor(
                        yn[:], y[:], nmu[:, 0:1],
                        rstd[:, 0:1].broadcast_to([128, C]),
                        TS.add, TS.mult)
                    yg = lp.tile([128, C], dt.float32, tag="yg", name="yg")
                    nc.vector.tensor_mul(yg[:], yn[:], grep)
                    ot = lp.tile([128, C], dt.float16, tag="ot", name="ot")
                    nc.vector.tensor_add(ot[:], yg[:], brep)
                    nc.sync.dma_start(out_h[nt * 128:(nt + 1) * 128, :], ot[:])

            for g in range(GPC):
                em_t = sp.tile([128, 16, K], dt.float16, tag="em", name="em_t")
                nc.sync.dma_start(
                    em_t[:], em_h[g * NPG:(g + 1) * NPG, :].rearrange(
                        "(j p) k -> p j k", p=128))
                xg_t = sp.tile([128, 16, C], dt.float16, tag="xg", name="xg_t")
                nc.sync.dma_start(
                    xg_t[:], x16_h[g * NPG:(g + 1) * NPG, :].rearrange(
                        "(j p) k -> p j k", p=128))
                evT_t = sp.tile([128, NPG], dt.float16, tag="evT", name="evT_t")
                nc.sync.dma_start(evT_t[:], evT_h[g * K:(g + 1) * K, :])
                dec_t = sp.tile([128, C], dt.float32, tag="dec", name="dec_t")
                nc.sync.dma_start(dec_t[:], dec_h[g])

                bc0 = int(cum_CB[g * BPG])
                bc1 = int(cum_CB[(g + 1) * BPG])
                band_t = sp.tile([128, bc1 - bc0, BW], dt.float8e4, tag="band",
                                 name="band_t")
                nc.sync.dma_start(band_t[:], band_h[:, bc0:bc1, :])

                pxs = pp.tile([128, C], dt.float32, tag="xspec", bufs=1,
                              name="pxs")
                for j in range(16):
                    nc.tensor.matmul(pxs[:], em_t[:, j, :], xg_t[:, j, :],
                                     start=(j == 0), stop=(j == 15))
                xsd = sp.tile([128, C], dt.float16, tag="xsd", name="xsd")
                nc.vector.tensor_mul(xsd[:], pxs[:], dec_t[:])

                diffT = [sp.tile([128, NPG], dt.float16, tag=f"diffT{h2}",
                                 name=f"diffT{h2}")
                         for h2 in range(2)]
                for h2 in range(2):
                    for j in range(4):
                        pd = pp.tile([128, 512], dt.float32, tag="diff", bufs=1,
                                     name="pd")
                        nc.tensor.matmul(pd[:],
                                         xsd[:, h2 * 128:(h2 + 1) * 128],
                                         evT_t[:, j * 512:(j + 1) * 512],
                                         start=True, stop=True)
                        nc.scalar.copy(diffT[h2][:, j * 512:(j + 1) * 512], pd[:])

                for b in range(BPG):
                    B = g * BPG + b
                    if b % 4 == 0:
                        segT_t = sg.tile([128, 2, 512], dt.float16, tag="segT",
                                         name="segT_t")
                    seg_block(B, band_t, bc0, segT_t)
                    if b % 4 == 3:
                        mlp_group(B, segT_t, diffT)
    _legalize_waits(nc)
    return nc


# ---------------------------------------------------------------- entry
def kernel(**inputs):
    in_maps, plan = _prepare(inputs)
    nc = _build(plan)
    res = bass_utils.run_bass_kernel_spmd(nc, in_maps,
                                          core_ids=list(range(NCORES)))
    return np.concatenate([res.results[i]["out"] for i in range(NCORES)],
                          0).astype(np.float32)


# revision 10
# speedup vs baseline: 1.0393x; 1.0393x over previous
"""DiffusionBlock TRN2 kernel: spectral diffusion + sparse COO gradient op +
MLP + residual LayerNorm, sharded over 8 NeuronCores by node rows.

v5: the sparse segment-sum runs as fp8 matmuls between a host-assembled
dense edge-source stream (xe[slot] = x[col_e], streamed at full DMA rate —
no per-edge gather descriptors) and narrow banded one-hot operands (edges
sorted by destination row within each 128-row block, vals baked in).

Self-contained: hardcodes all shapes; builds + compiles a Bass program at
call time (specialized to the edge distribution), runs SPMD on cores 0-7.
"""
import sys
sys.path.insert(0, '/opt/trn_rl_repo')
import numpy as np
import ml_dtypes
import concourse.mybir as mybir
from concourse.bass import Bass
from concourse.tile import TileContext
from concourse import bass_utils

dt = mybir.dt
FP8 = ml_dtypes.float8_e4m3

# problem dims (hardcoded per contract)
N, C, K, G, E = 65536, 256, 128, 32, 2097152
LN_EPS = 1e-5
NCORES = 8
RPC = N // NCORES          # rows per core = 8192
GPC = G // NCORES          # graphs per core = 4
NPG = N // G               # nodes per graph = 2048
NBLK = RPC // 128          # 128-row blocks per core = 64
BPG = NPG // 128           # blocks per graph = 16


# ---------------------------------------------------------------- BIR fixups
_wspill = [0]


def _legalize_waits(nc):
    """This walrus accepts at most 1 sync-wait per instruction (2 for
    EventSemaphore). Spill extras into EventSemaphore insts inserted just
    before, same engine. Also run codegen_inst_isa_subclasses (Bacc does it,
    raw Bass doesn't) so extended-ISA insts get their raw words."""
    mybir.codegen_inst_isa_subclasses(nc)
    f = nc.m.functions[0]
    for bb in f.blocks:
        out = []
        changed = False
        for ins in bb.instructions:
            si = ins.sync_info
            cap = 2 if ins.opcode == 'EventSemaphore' else 1
            if si is not None and si.on_wait is not None and len(si.on_wait) > cap:
                waits = list(si.on_wait)
                keep, spill = waits[:cap], waits[cap:]
                while spill:
                    batch, spill = spill[:2], spill[2:]
                    _wspill[0] += 1
                    es = mybir.InstEventSemaphore(
                        name=f"WSPILL-{_wspill[0]}", ins=[], outs=[])
                    es.engine = ins.engine
                    es.sync_info = mybir.SyncInfo(on_wait=batch, on_update=[])
                    out.append(es)
                si.on_wait = keep
                changed = True
            out.append(ins)
        if changed:
            bb.instructions = out
    return nc


# ---------------------------------------------------------------- host prep
def _prepare(inputs):
    x = np.asarray(inputs["x"], np.float32)
    evals = np.asarray(inputs["evals_batch"], np.float32)
    evecs = np.asarray(inputs["evecs"], np.float32)
    mass = np.asarray(inputs["mass"], np.float32)
    row = np.asarray(inputs["row"]).astype(np.int64)
    col = np.asarray(inputs["col"]).astype(np.int64)
    vals = np.asarray(inputs["vals"], np.float32)
    t_params = np.asarray(inputs["t_params"], np.float32)
    grad_W = np.asarray(inputs["grad_W"], np.float32)
    grad_b = np.asarray(inputs["grad_b"], np.float32)
    W1 = np.asarray(inputs["W1"], np.float32)
    b1 = np.asarray(inputs["b1"], np.float32)
    W2 = np.asarray(inputs["W2"], np.float32)
    b2 = np.asarray(inputs["b2"], np.float32)
    ln_g = np.asarray(inputs["ln_g"], np.float32)
    ln_b = np.asarray(inputs["ln_b"], np.float32)

    xf8_full = x.astype(FP8)
    x16_full = x.astype(np.float16)

    # fold grad_W / grad_b into the second half of W1 (host, fp64 for accuracy)
    W1a = W1[:, :C]
    W1b = W1[:, C:]
    Wfold = (W1b.astype(np.float64) @ grad_W.astype(np.float64)).astype(np.float32)
    b1f_np = b1 + (W1b.astype(np.float64) @ grad_b.astype(np.float64)).astype(np.float32)

    # decay[g,k,c] = exp(-|t_c| * max(ev_gk, 0))
    t = np.abs(t_params)
    ev = np.maximum(evals.reshape(G, K), 0.0)
    decay = np.exp(-ev[:, :, None] * t[None, None, :]).astype(np.float32)  # [G,K,C]

    em_full = (evecs * mass[:, None]).astype(np.float16)   # [N,K]
    ev16_full = evecs.astype(np.float16)

    # ---- edge partitioning: per core, per 128-row dest block, sorted by
    # lrow (dest row within block). Chunks split at lrow 64-boundaries so
    # each banded one-hot is a 64-wide PSUM window at offset {0,64}
    # (PE tile_position: 32-aligned, quadrant 3 unusable on trn2). ----
    BW = 64
    core_of = row >> 13               # row // 8192
    percore = []
    # counts per (core, block, quarter)
    counts_q = np.zeros((NCORES, NBLK, 2), np.int64)
    for i in range(NCORES):
        sel = np.where(core_of == i)[0]
        r = row[sel] - i * RPC
        c_ = col[sel]
        v = vals[sel]
        blk = r >> 7
        lrow = r & 127
        order = np.lexsort((lrow, blk))
        blk_s = blk[order]
        lrow_s = lrow[order]
        percore.append((blk_s, lrow_s, c_[order], v[order]))
        np.add.at(counts_q[i], (blk_s, lrow_s >> 6), 1)

    # uniform chunk counts across cores (SPMD: one program)
    CBq = ((counts_q.max(0) + 127) // 128).astype(np.int64)  # [NBLK, 2]
    CB = CBq.sum(1)                                          # chunks per block
    cum_CB = np.concatenate([[0], np.cumsum(CB)]).astype(np.int64)
    CTsum = int(cum_CB[-1])
    # chunk id -> psum window offset (64 * half)
    cum_CBq = np.concatenate([[0], np.cumsum(CBq.reshape(-1))]).astype(np.int64)
    off_chunk = np.zeros(CTsum, np.int64)
    for b in range(NBLK):
        for q in range(2):
            s = int(cum_CBq[b * 2 + q])
            e = int(cum_CBq[b * 2 + q + 1])
            off_chunk[s:e] = 64 * q

    in_maps = []
    for i in range(NCORES):
        blk_s, lrow_s, c_s, v_s = percore[i]
        half = lrow_s >> 6
        grp = blk_s * 2 + half                   # sorted ascending
        gstart_core = np.concatenate(
            [[0], np.cumsum(np.bincount(grp, minlength=NBLK * 2))])
        pos = np.arange(len(grp)) - gstart_core[grp]
        chunk = pos >> 7
        slot = pos & 127
        cid = cum_CBq[grp] + chunk

        xe = np.zeros((128, CTsum, C), FP8)
        xe[slot, cid, :] = xf8_full[c_s]
        oh0 = np.zeros((128, NBLK, 128), FP8)
        m0 = cid == cum_CB[blk_s]                # first chunk of the block
        oh0[slot[m0], blk_s[m0], lrow_s[m0]] = v_s[m0]
        band = np.zeros((128, CTsum, BW), FP8)
        mb = ~m0
        rel = lrow_s[mb] - off_chunk[cid[mb]]
        assert rel.min() >= 0 and rel.max() < BW
        band[slot[mb], cid[mb], rel] = v_s[mb]

        sl_rows = slice(i * RPC, (i + 1) * RPC)
        sl_g = slice(i * GPC, (i + 1) * GPC)
        evT16 = np.ascontiguousarray(
            ev16_full[sl_rows].reshape(GPC, NPG, K).transpose(0, 2, 1)
        ).reshape(GPC * K, NPG)

        c16 = np.zeros((128, 1664), np.float16)
        off = 0
        W1aT = W1a.T.astype(np.float16)
        WfT = Wfold.T.astype(np.float16)
        for Wt in (W1aT, WfT):
            for k in range(2):
                for m in range(2):
                    c16[:, off:off + 128] = Wt[k * 128:(k + 1) * 128,
                                               m * 128:(m + 1) * 128]
                    off += 128
        W2T = W2.T.astype(np.float16)
        for k in range(2):
            c16[:, off:off + 256] = W2T[k * 128:(k + 1) * 128, :]
            off += 256
        c16[:, off:off + 128] = np.eye(128, dtype=np.float16)
        off += 128
        assert off == 1664

        c32 = np.zeros((128, 772), np.float32)
        c32[:, 0] = b1f_np[:128]
        c32[:, 1] = b1f_np[128:]
        c32[:, 2:258] = np.broadcast_to(ln_g, (128, C))
        c32[:, 258:514] = np.broadcast_to(ln_b, (128, C))
        c32[:, 514] = LN_EPS
        c32[:, 516:772] = np.broadcast_to(b2, (128, C))

        in_maps.append({
            "xe": xe,
            "oh0": oh0,
            "band": band,
            "x16": np.ascontiguousarray(x16_full[sl_rows]),
            "em16": np.ascontiguousarray(em_full[sl_rows]),
            "evT16": evT16,
            "decay": np.ascontiguousarray(decay[sl_g]),
            "c16": c16,
            "c32": c32,
        })
    plan = dict(CB=CB, CBq=CBq, cum_CB=cum_CB, cum_CBq=cum_CBq,
                CTsum=CTsum, BW=BW, off_chunk=off_chunk)
    return in_maps, plan


# ---------------------------------------------------------------- program
def _build(plan):
    CB, cum_CB = plan["CB"], plan["cum_CB"]
    CBq, cum_CBq = plan["CBq"], plan["cum_CBq"]
    CTsum, BW = plan["CTsum"], plan["BW"]
    off_chunk = plan["off_chunk"]

    nc = Bass()
    xe_h = nc.dram_tensor("xe", [128, CTsum, C], dt.float8e4,
                          kind="ExternalInput")
    oh0_h = nc.dram_tensor("oh0", [128, NBLK, 128], dt.float8e4,
                           kind="ExternalInput")
    band_h = nc.dram_tensor("band", [128, CTsum, BW], dt.float8e4,
                            kind="ExternalInput")
    x16_h = nc.dram_tensor("x16", [RPC, C], dt.float16, kind="ExternalInput")
    em_h = nc.dram_tensor("em16", [RPC, K], dt.float16, kind="ExternalInput")
    evT_h = nc.dram_tensor("evT16", [GPC * K, NPG], dt.float16, kind="ExternalInput")
    dec_h = nc.dram_tensor("decay", [GPC, K, C], dt.float32, kind="ExternalInput")
    c16_h = nc.dram_tensor("c16", [128, 1664], dt.float16, kind="ExternalInput")
    c32_h = nc.dram_tensor("c32", [128, 772], dt.float32, kind="ExternalInput")
    out_h = nc.dram_tensor("out", [RPC, C], dt.float16, kind="ExternalOutput")

    TS = mybir.AluOpType
    AF = mybir.ActivationFunctionType

    with TileContext(nc) as tc:
        with tc.tile_pool(name="consts", bufs=1) as cp, \
             tc.tile_pool(name="spec", bufs=2) as sp, \
             tc.tile_pool(name="edge", bufs=2) as ep, \
             tc.tile_pool(name="segp", bufs=2) as sg, \
             tc.tile_pool(name="mlp", bufs=2) as mp, \
             tc.tile_pool(name="ln", bufs=3) as lp, \
             tc.tile_pool(name="ps", bufs=1, space="PSUM") as pp:
            c16 = cp.tile([128, 1664], dt.float16)
            c32 = cp.tile([128, 772], dt.float32)
            nc.sync.dma_start(c16[:], c16_h[:, :])
            nc.sync.dma_start(c32[:], c32_h[:, :])
            oh0_t = cp.tile([128, NBLK, 128], dt.float8e4)
            nc.sync.dma_start(oh0_t[:], oh0_h[:, :, :])
            W1aT = [[c16[:, (k * 2 + m) * 128:(k * 2 + m + 1) * 128]
                     for m in range(2)] for k in range(2)]
            WfT = [[c16[:, 512 + (k * 2 + m) * 128:512 + (k * 2 + m + 1) * 128]
                    for m in range(2)] for k in range(2)]
            W2T = [c16[:, 1024 + k * 256:1024 + (k + 1) * 256] for k in range(2)]
            ident = c16[:, 1536:1664]
            b1f = [c32[:, m:m + 1] for m in range(2)]
            grep = c32[:, 2:258]
            brep = c32[:, 258:514]
            eps_ap = c32[:, 514:515]
            zero_ap = c32[:, 515:516]
            b2rep = c32[:, 516:772]

            def seg_block(B, band_t, bc0, segT_t):
                nch = int(CB[B])
                cid0 = int(cum_CB[B])
                xe_t = ep.tile([128, nch, C], dt.float8e4, tag="xe", bufs=4,
                               name="xe_t")
                nc.sync.dma_start(xe_t[:],
                                  xe_h[:, cid0:cid0 + nch, :])
                psg = pp.tile([128, C], dt.float32, tag="grad", bufs=2,
                              name="psg")
                nc.tensor.matmul(psg[:], oh0_t[:, B, :], xe_t[:, 0, :],
                                 start=True, stop=(nch == 1))
                for j in range(1, nch):
                    o = int(off_chunk[cid0 + j])
                    nc.tensor.matmul(psg[o:o + BW, :],
                                     band_t[:, cid0 - bc0 + j, :],
                                     xe_t[:, j, :],
                                     start=False, stop=(j == nch - 1))
                segNM = sg.tile([128, C], dt.float16, tag="segNM", name="segNM")
                nc.scalar.copy(segNM[:], psg[:])
                for h2 in range(2):
                    tp = pp.tile([128, 128], dt.float16, tag="tp", bufs=1,
                                 name="tp")
                    nc.tensor.transpose(tp[:], segNM[:, h2 * 128:(h2 + 1) * 128],
                                        ident)
                    nc.vector.tensor_copy(
                        segT_t[:, h2, (B % 4) * 128:(B % 4 + 1) * 128], tp[:])

            def mlp_group(B, segT_t, diffT, xg_t):
                n0 = (B - 3) * 128
                goff = n0 % NPG
                gslice = slice(goff, goff + 512)
                hT = mp.tile([128, 2, 512], dt.float16, tag="hT", name="hT")
                for m in range(2):
                    ph = pp.tile([128, 512], dt.float32, tag="h", bufs=2,
                                 name="ph")
                    nc.tensor.matmul(ph[:], W1aT[0][m], diffT[0][:, gslice],
                                     start=True, stop=False)
                    nc.tensor.matmul(ph[:], W1aT[1][m], diffT[1][:, gslice],
                                     start=False, stop=False)
                    for k in range(2):
                        nc.tensor.matmul(ph[:], WfT[k][m], segT_t[:, k, :],
                                         start=False, stop=(k == 1))
                    nc.scalar.activation(hT[:, m, :], ph[:], AF.Relu,
                                         bias=b1f[m], scale=1.0)
                for tt in range(4):
                    nt = B - 3 + tt
                    py = pp.tile([128, C], dt.float32, tag="y", bufs=1,
                                 name="py")
                    for k in range(2):
                        nc.tensor.matmul(py[:], hT[:, k, tt * 128:(tt + 1) * 128],
                                         W2T[k], start=(k == 0), stop=(k == 1))
                    y0 = lp.tile([128, C], dt.float32, tag="y0", name="y0")
                    nc.vector.tensor_add(y0[:], py[:], xg_t[:, nt % 16, :])
                    y = lp.tile([128, C], dt.float32, tag="y", name="y")
                    nc.vector.tensor_add(y[:], y0[:], b2rep)
                    nsum = lp.tile([128, 1], dt.float32, tag="s0", name="s0")
                    nc.vector.tensor_reduce(nsum[:], y[:], mybir.AxisListType.X,
                                            TS.add, negate=True)
                    nmu = lp.tile([128, 1], dt.float32, tag="s1", name="s1")
                    nc.vector.tensor_scalar_mul(nmu[:], nsum[:], 1.0 / C)
                    sq = lp.tile([128, C], dt.float32, tag="sq", name="sq")
                    sqs = lp.tile([128, 1], dt.float32, tag="s2", name="s2")
                    nc.scalar.activation(sq[:], y[:], AF.Square,
                                         bias=zero_ap, accum_out=sqs[:])
                    ex2 = lp.tile([128, 1], dt.float32, tag="s3", name="s3")
                    nc.vector.tensor_scalar_mul(ex2[:], sqs[:], 1.0 / C)
                    mu2 = lp.tile([128, 1], dt.float32, tag="s4", name="s4")
                    nc.vector.tensor_mul(mu2[:], nmu[:], nmu[:])
                    var = lp.tile([128, 1], dt.float32, tag="s5", name="s5")
                    nc.vector.tensor_sub(var[:], ex2[:], mu2[:])
                    sd = lp.tile([128, 1], dt.float32, tag="s6", name="s6")
                    nc.scalar.activation(sd[:], var[:], AF.Sqrt, bias=eps_ap)
                    rstd = lp.tile([128, 1], dt.float32, tag="s7", name="s7")
                    nc.vector.reciprocal(rstd[:], sd[:])
                    yn = lp.tile([128, C], dt.float32, tag="yn", name="yn")
                    nc.vector.scalar_tensor_tensor(
                        yn[:], y[:], nmu[:, 0:1],
                        rstd[:, 0:1].broadcast_to([128, C]),
                        TS.add, TS.mult)
                    yg = lp.tile([128, C], dt.float32, tag="yg", name="yg")
                    nc.vector.tensor_mul(yg[:], yn[:], grep)
                    ot = lp.tile([128, C], dt.float16, tag="ot", name="ot")
                    nc.vector.tensor_add(ot[:], yg[:], brep)
                    nc.sync.dma_start(out_h[nt * 128:(nt + 1) * 128, :], ot[:])

            for g in range(GPC):
                em_t = sp.tile([128, 16, K], dt.float16, tag="em", name="em_t")
                nc.sync.dma_start(
                    em_t[:], em_h[g * NPG:(g + 1) * NPG, :].rearrange(
                        "(j p) k -> p j k", p=128))
                xg_t = sp.tile([128, 16, C], dt.float16, tag="xg", name="xg_t")
                nc.sync.dma_start(
                    xg_t[:], x16_h[g * NPG:(g + 1) * NPG, :].rearrange(
                        "(j p) k -> p j k", p=128))
                evT_t = sp.tile([128, NPG], dt.float16, tag="evT", name="evT_t")
                nc.sync.dma_start(evT_t[:], evT_h[g * K:(g + 1) * K, :])
                dec_t = sp.tile([128, C], dt.float32, tag="dec", name="dec_t")
                nc.sync.dma_start(dec_t[:], dec_h[g])

                bc0 = int(cum_CB[g * BPG])
                bc1 = int(cum_CB[(g + 1) * BPG])
                band_t = sp.tile([128, bc1 - bc0, BW], dt.float8e4, tag="band",
                                 name="band_t")
                nc.sync.dma_start(band_t[:], band_h[:, bc0:bc1, :])

                pxs = pp.tile([128, C], dt.float32, tag="xspec", bufs=1,
                              name="pxs")
                for j in range(16):
                    nc.tensor.matmul(pxs[:], em_t[:, j, :], xg_t[:, j, :],
                                     start=(j == 0), stop=(j == 15))
                xsd = sp.tile([128, C], dt.float16, tag="xsd", name="xsd")
                nc.vector.tensor_mul(xsd[:], pxs[:], dec_t[:])

                diffT = [sp.tile([128, NPG], dt.float16, tag=f"diffT{h2}",
                                 name=f"diffT{h2}")
                         for h2 in range(2)]
                for h2 in range(2):
                    for j in range(4):
                        pd = pp.tile([128, 512], dt.float32, tag="diff", bufs=1,
                                     name="pd")
                        nc.tensor.matmul(pd[:],
                                         xsd[:, h2 * 128:(h2 + 1) * 128],
                                         evT_t[:, j * 512:(j + 1) * 512],
                                         start=True, stop=True)
                        nc.scalar.copy(diffT[h2][:, j * 512:(j + 1) * 512], pd[:])

                for b in range(BPG):
                    B = g * BPG + b
                    if b % 4 == 0:
                        segT_t = sg.tile([128, 2, 512], dt.float16, tag="segT",
                                         name="segT_t")
                    seg_block(B, band_t, bc0, segT_t)
                    if b % 4 == 3:
                        mlp_group(B, segT_t, diffT, xg_t)
    _legalize_waits(nc)
    return nc


# ---------------------------------------------------------------- entry
def kernel(**inputs):
    in_maps, plan = _prepare(inputs)
    nc = _build(plan)
    res = bass_utils.run_bass_kernel_spmd(nc, in_maps,
                                          core_ids=list(range(NCORES)))
    return np.concatenate([res.results[i]["out"] for i in range(NCORES)],
                          0).astype(np.float32)


# revision 12
# speedup vs baseline: 1.0682x; 1.0278x over previous
"""DiffusionBlock TRN2 kernel: spectral diffusion + sparse COO gradient op +
MLP + residual LayerNorm, sharded over 8 NeuronCores by node rows.

v5: the sparse segment-sum runs as fp8 matmuls between a host-assembled
dense edge-source stream (xe[slot] = x[col_e], streamed at full DMA rate —
no per-edge gather descriptors) and narrow banded one-hot operands (edges
sorted by destination row within each 128-row block, vals baked in).

Self-contained: hardcodes all shapes; builds + compiles a Bass program at
call time (specialized to the edge distribution), runs SPMD on cores 0-7.
"""
import sys
sys.path.insert(0, '/opt/trn_rl_repo')
import numpy as np
import ml_dtypes
import concourse.mybir as mybir
from concourse.bass import Bass
from concourse.tile import TileContext
from concourse import bass_utils

dt = mybir.dt
FP8 = ml_dtypes.float8_e4m3

# problem dims (hardcoded per contract)
N, C, K, G, E = 65536, 256, 128, 32, 2097152
LN_EPS = 1e-5
NCORES = 8
RPC = N // NCORES          # rows per core = 8192
GPC = G // NCORES          # graphs per core = 4
NPG = N // G               # nodes per graph = 2048
NBLK = RPC // 128          # 128-row blocks per core = 64
BPG = NPG // 128           # blocks per graph = 16


# ---------------------------------------------------------------- BIR fixups
_wspill = [0]


def _legalize_waits(nc):
    """This walrus accepts at most 1 sync-wait per instruction (2 for
    EventSemaphore). Spill extras into EventSemaphore insts inserted just
    before, same engine. Also run codegen_inst_isa_subclasses (Bacc does it,
    raw Bass doesn't) so extended-ISA insts get their raw words."""
    mybir.codegen_inst_isa_subclasses(nc)
    f = nc.m.functions[0]
    for bb in f.blocks:
        out = []
        changed = False
        for ins in bb.instructions:
            si = ins.sync_info
            cap = 2 if ins.opcode == 'EventSemaphore' else 1
            if si is not None and si.on_wait is not None and len(si.on_wait) > cap:
                waits = list(si.on_wait)
                keep, spill = waits[:cap], waits[cap:]
                while spill:
                    batch, spill = spill[:2], spill[2:]
                    _wspill[0] += 1
                    es = mybir.InstEventSemaphore(
                        name=f"WSPILL-{_wspill[0]}", ins=[], outs=[])
                    es.engine = ins.engine
                    es.sync_info = mybir.SyncInfo(on_wait=batch, on_update=[])
                    out.append(es)
                si.on_wait = keep
                changed = True
            out.append(ins)
        if changed:
            bb.instructions = out
    return nc


# ---------------------------------------------------------------- host prep
def _prepare(inputs):
    x = np.asarray(inputs["x"], np.float32)
    evals = np.asarray(inputs["evals_batch"], np.float32)
    evecs = np.asarray(inputs["evecs"], np.float32)
    mass = np.asarray(inputs["mass"], np.float32)
    row = np.asarray(inputs["row"]).astype(np.int64)
    col = np.asarray(inputs["col"]).astype(np.int64)
    vals = np.asarray(inputs["vals"], np.float32)
    t_params = np.asarray(inputs["t_params"], np.float32)
    grad_W = np.asarray(inputs["grad_W"], np.float32)
    grad_b = np.asarray(inputs["grad_b"], np.float32)
    W1 = np.asarray(inputs["W1"], np.float32)
    b1 = np.asarray(inputs["b1"], np.float32)
    W2 = np.asarray(inputs["W2"], np.float32)
    b2 = np.asarray(inputs["b2"], np.float32)
    ln_g = np.asarray(inputs["ln_g"], np.float32)
    ln_b = np.asarray(inputs["ln_b"], np.float32)

    xf8_full = x.astype(FP8)
    x16_full = x.astype(np.float16)

    # fold grad_W / grad_b into the second half of W1 (host, fp64 for accuracy)
    W1a = W1[:, :C]
    W1b = W1[:, C:]
    Wfold = (W1b.astype(np.float64) @ grad_W.astype(np.float64)).astype(np.float32)
    b1f_np = b1 + (W1b.astype(np.float64) @ grad_b.astype(np.float64)).astype(np.float32)

    # decay[g,k,c] = exp(-|t_c| * max(ev_gk, 0))
    t = np.abs(t_params)
    ev = np.maximum(evals.reshape(G, K), 0.0)
    decay = np.exp(-ev[:, :, None] * t[None, None, :]).astype(np.float32)  # [G,K,C]

    em_full = (evecs * mass[:, None]).astype(np.float16)   # [N,K]
    ev16_full = evecs.astype(np.float16)

    # ---- edge partitioning: per core, per 128-row dest block, sorted by
    # lrow (dest row within block). Chunks split at lrow 64-boundaries so
    # each banded one-hot is a 64-wide PSUM window at offset {0,64}
    # (PE tile_position: 32-aligned, quadrant 3 unusable on trn2). ----
    BW = 64
    core_of = row >> 13               # row // 8192
    percore = []
    # counts per (core, block, quarter)
    counts_q = np.zeros((NCORES, NBLK, 2), np.int64)
    for i in range(NCORES):
        sel = np.where(core_of == i)[0]
        r = row[sel] - i * RPC
        c_ = col[sel]
        v = vals[sel]
        blk = r >> 7
        lrow = r & 127
        order = np.lexsort((lrow, blk))
        blk_s = blk[order]
        lrow_s = lrow[order]
        percore.append((blk_s, lrow_s, c_[order], v[order]))
        np.add.at(counts_q[i], (blk_s, lrow_s >> 6), 1)

    # uniform chunk counts across cores (SPMD: one program)
    CBq = ((counts_q.max(0) + 127) // 128).astype(np.int64)  # [NBLK, 2]
    CB = CBq.sum(1)                                          # chunks per block
    cum_CB = np.concatenate([[0], np.cumsum(CB)]).astype(np.int64)
    CTsum = int(cum_CB[-1])
    # chunk id -> psum window offset (64 * half)
    cum_CBq = np.concatenate([[0], np.cumsum(CBq.reshape(-1))]).astype(np.int64)
    off_chunk = np.zeros(CTsum, np.int64)
    for b in range(NBLK):
        for q in range(2):
            s = int(cum_CBq[b * 2 + q])
            e = int(cum_CBq[b * 2 + q + 1])
            off_chunk[s:e] = 64 * q

    in_maps = []
    for i in range(NCORES):
        blk_s, lrow_s, c_s, v_s = percore[i]
        half = lrow_s >> 6
        grp = blk_s * 2 + half                   # sorted ascending
        gstart_core = np.concatenate(
            [[0], np.cumsum(np.bincount(grp, minlength=NBLK * 2))])
        pos = np.arange(len(grp)) - gstart_core[grp]
        chunk = pos >> 7
        slot = pos & 127
        cid = cum_CBq[grp] + chunk

        xe = np.zeros((128, CTsum, C), FP8)
        xe[slot, cid, :] = xf8_full[c_s]
        oh0 = np.zeros((128, NBLK, 128), FP8)
        m0 = cid == cum_CB[blk_s]                # first chunk of the block
        oh0[slot[m0], blk_s[m0], lrow_s[m0]] = v_s[m0]
        band = np.zeros((128, CTsum, BW), FP8)
        mb = ~m0
        rel = lrow_s[mb] - off_chunk[cid[mb]]
        assert rel.min() >= 0 and rel.max() < BW
        band[slot[mb], cid[mb], rel] = v_s[mb]

        sl_rows = slice(i * RPC, (i + 1) * RPC)
        sl_g = slice(i * GPC, (i + 1) * GPC)
        evT16 = np.ascontiguousarray(
            ev16_full[sl_rows].reshape(GPC, NPG, K).transpose(0, 2, 1)
        ).reshape(GPC * K, NPG)

        c16 = np.zeros((128, 1664), np.float16)
        off = 0
        W1aT = W1a.T.astype(np.float16)
        WfT = Wfold.T.astype(np.float16)
        for Wt in (W1aT, WfT):
            for k in range(2):
                for m in range(2):
                    c16[:, off:off + 128] = Wt[k * 128:(k + 1) * 128,
                                               m * 128:(m + 1) * 128]
                    off += 128
        W2T = W2.T.astype(np.float16)
        for k in range(2):
            c16[:, off:off + 256] = W2T[k * 128:(k + 1) * 128, :]
            off += 256
        c16[:, off:off + 128] = np.eye(128, dtype=np.float16)
        off += 128
        assert off == 1664

        c32 = np.zeros((128, 772), np.float32)
        c32[:, 0] = b1f_np[:128]
        c32[:, 1] = b1f_np[128:]
        c32[:, 2:258] = np.broadcast_to(ln_g, (128, C))
        c32[:, 258:514] = np.broadcast_to(ln_b, (128, C))
        c32[:, 514] = LN_EPS
        c32[:, 516:772] = np.broadcast_to(b2, (128, C))

        in_maps.append({
            "xe": xe,
            "oh0": oh0,
            "band": band,
            "x16": np.ascontiguousarray(x16_full[sl_rows]),
            "em16": np.ascontiguousarray(em_full[sl_rows]),
            "evT16": evT16,
            "decay": np.ascontiguousarray(decay[sl_g]),
            "c16": c16,
            "c32": c32,
        })
    plan = dict(CB=CB, CBq=CBq, cum_CB=cum_CB, cum_CBq=cum_CBq,
                CTsum=CTsum, BW=BW, off_chunk=off_chunk)
    return in_maps, plan


# ---------------------------------------------------------------- program
def _build(plan):
    CB, cum_CB = plan["CB"], plan["cum_CB"]
    CBq, cum_CBq = plan["CBq"], plan["cum_CBq"]
    CTsum, BW = plan["CTsum"], plan["BW"]
    off_chunk = plan["off_chunk"]

    nc = Bass()
    xe_h = nc.dram_tensor("xe", [128, CTsum, C], dt.float8e4,
                          kind="ExternalInput")
    oh0_h = nc.dram_tensor("oh0", [128, NBLK, 128], dt.float8e4,
                           kind="ExternalInput")
    band_h = nc.dram_tensor("band", [128, CTsum, BW], dt.float8e4,
                            kind="ExternalInput")
    x16_h = nc.dram_tensor("x16", [RPC, C], dt.float16, kind="ExternalInput")
    em_h = nc.dram_tensor("em16", [RPC, K], dt.float16, kind="ExternalInput")
    evT_h = nc.dram_tensor("evT16", [GPC * K, NPG], dt.float16, kind="ExternalInput")
    dec_h = nc.dram_tensor("decay", [GPC, K, C], dt.float32, kind="ExternalInput")
    c16_h = nc.dram_tensor("c16", [128, 1664], dt.float16, kind="ExternalInput")
    c32_h = nc.dram_tensor("c32", [128, 772], dt.float32, kind="ExternalInput")
    out_h = nc.dram_tensor("out", [RPC, C], dt.float16, kind="ExternalOutput")

    TS = mybir.AluOpType
    AF = mybir.ActivationFunctionType

    with TileContext(nc) as tc:
        with tc.tile_pool(name="consts", bufs=1) as cp, \
             tc.tile_pool(name="spec", bufs=2) as sp, \
             tc.tile_pool(name="edge", bufs=2) as ep, \
             tc.tile_pool(name="segp", bufs=2) as sg, \
             tc.tile_pool(name="mlp", bufs=2) as mp, \
             tc.tile_pool(name="ln", bufs=3) as lp, \
             tc.tile_pool(name="ps", bufs=1, space="PSUM") as pp:
            c16 = cp.tile([128, 1664], dt.float16)
            c32 = cp.tile([128, 772], dt.float32)
            nc.sync.dma_start(c16[:], c16_h[:, :])
            nc.sync.dma_start(c32[:], c32_h[:, :])
            oh0_t = cp.tile([128, NBLK, 128], dt.float8e4)
            nc.sync.dma_start(oh0_t[:], oh0_h[:, :, :])
            W1aT = [[c16[:, (k * 2 + m) * 128:(k * 2 + m + 1) * 128]
                     for m in range(2)] for k in range(2)]
            WfT = [[c16[:, 512 + (k * 2 + m) * 128:512 + (k * 2 + m + 1) * 128]
                    for m in range(2)] for k in range(2)]
            W2T = [c16[:, 1024 + k * 256:1024 + (k + 1) * 256] for k in range(2)]
            ident = c16[:, 1536:1664]
            b1f = [c32[:, m:m + 1] for m in range(2)]
            grep = c32[:, 2:258]
            brep = c32[:, 258:514]
            eps_ap = c32[:, 514:515]
            zero_ap = c32[:, 515:516]
            b2rep = c32[:, 516:772]

            def seg_block(B, band_t, bc0, segT_t):
                nch = int(CB[B])
                cid0 = int(cum_CB[B])
                xe_t = ep.tile([128, nch, C], dt.float8e4, tag="xe", bufs=4,
                               name="xe_t")
                nc.sync.dma_start(xe_t[:],
                                  xe_h[:, cid0:cid0 + nch, :])
                psg = pp.tile([128, C], dt.float32, tag="grad", bufs=2,
                              name="psg")
                nc.tensor.matmul(psg[:], oh0_t[:, B, :], xe_t[:, 0, :],
                                 start=True, stop=(nch == 1))
                for j in range(1, nch):
                    o = int(off_chunk[cid0 + j])
                    nc.tensor.matmul(psg[o:o + BW, :],
                                     band_t[:, cid0 - bc0 + j, :],
                                     xe_t[:, j, :],
                                     start=False, stop=(j == nch - 1))
                segNM = sg.tile([128, C], dt.float16, tag="segNM", name="segNM")
                nc.scalar.copy(segNM[:], psg[:])
                for h2 in range(2):
                    tp = pp.tile([128, 128], dt.float16, tag="tp", bufs=2,
                                 name="tp")
                    nc.tensor.transpose(tp[:], segNM[:, h2 * 128:(h2 + 1) * 128],
                                        ident)
                    nc.vector.tensor_copy(
                        segT_t[:, h2, (B % 4) * 128:(B % 4 + 1) * 128], tp[:])

            def mlp_group(B, segT_t, diffT, xg_t):
                n0 = (B - 3) * 128
                goff = n0 % NPG
                gslice = slice(goff, goff + 512)
                hT = mp.tile([128, 2, 512], dt.float16, tag="hT", name="hT")
                for m in range(2):
                    ph = pp.tile([128, 512], dt.float32, tag="w512", bufs=2,
                                 name="ph")
                    nc.tensor.matmul(ph[:], W1aT[0][m], diffT[0][:, gslice],
                                     start=True, stop=False)
                    nc.tensor.matmul(ph[:], W1aT[1][m], diffT[1][:, gslice],
                                     start=False, stop=False)
                    for k in range(2):
                        nc.tensor.matmul(ph[:], WfT[k][m], segT_t[:, k, :],
                                         start=False, stop=(k == 1))
                    nc.scalar.activation(hT[:, m, :], ph[:], AF.Relu,
                                         bias=b1f[m], scale=1.0)
                for tt in range(4):
                    nt = B - 3 + tt
                    py = pp.tile([128, C], dt.float32, tag="p256", bufs=2,
                                 name="py")
                    for k in range(2):
                        nc.tensor.matmul(py[:], hT[:, k, tt * 128:(tt + 1) * 128],
                                         W2T[k], start=(k == 0), stop=(k == 1))
                    y0 = lp.tile([128, C], dt.float32, tag="y0", name="y0")
                    nc.vector.tensor_add(y0[:], py[:], xg_t[:, nt % 16, :])
                    y = lp.tile([128, C], dt.float32, tag="y", name="y")
                    nc.vector.tensor_add(y[:], y0[:], b2rep)
                    nsum = lp.tile([128, 1], dt.float32, tag="s0", name="s0")
                    nc.vector.tensor_reduce(nsum[:], y[:], mybir.AxisListType.X,
                                            TS.add, negate=True)
                    nmu = lp.tile([128, 1], dt.float32, tag="s1", name="s1")
                    nc.vector.tensor_scalar_mul(nmu[:], nsum[:], 1.0 / C)
                    sq = lp.tile([128, C], dt.float32, tag="sq", name="sq")
                    sqs = lp.tile([128, 1], dt.float32, tag="s2", name="s2")
                    nc.scalar.activation(sq[:], y[:], AF.Square,
                                         bias=zero_ap, accum_out=sqs[:])
                    ex2 = lp.tile([128, 1], dt.float32, tag="s3", name="s3")
                    nc.vector.tensor_scalar_mul(ex2[:], sqs[:], 1.0 / C)
                    mu2 = lp.tile([128, 1], dt.float32, tag="s4", name="s4")
                    nc.vector.tensor_mul(mu2[:], nmu[:], nmu[:])
                    var = lp.tile([128, 1], dt.float32, tag="s5", name="s5")
                    nc.vector.tensor_sub(var[:], ex2[:], mu2[:])
                    sd = lp.tile([128, 1], dt.float32, tag="s6", name="s6")
                    nc.scalar.activation(sd[:], var[:], AF.Sqrt, bias=eps_ap)
                    rstd = lp.tile([128, 1], dt.float32, tag="s7", name="s7")
                    nc.vector.reciprocal(rstd[:], sd[:])
                    yn = lp.tile([128, C], dt.float32, tag="yn", name="yn")
                    nc.vector.scalar_tensor_tensor(
                        yn[:], y[:], nmu[:, 0:1],
                        rstd[:, 0:1].broadcast_to([128, C]),
                        TS.add, TS.mult)
                    yg = lp.tile([128, C], dt.float32, tag="yg", name="yg")
                    nc.vector.tensor_mul(yg[:], yn[:], grep)
                    ot = lp.tile([128, C], dt.float16, tag="ot", name="ot")
                    nc.vector.tensor_add(ot[:], yg[:], brep)
                    nc.sync.dma_start(out_h[nt * 128:(nt + 1) * 128, :], ot[:])

            def load_graph(g):
                em_t = sp.tile([128, 16, K], dt.float16, tag="em", name="em_t")
                nc.sync.dma_start(
                    em_t[:], em_h[g * NPG:(g + 1) * NPG, :].rearrange(
                        "(j p) k -> p j k", p=128))
                xg_t = sp.tile([128, 16, C], dt.float16, tag="xg", name="xg_t")
                nc.sync.dma_start(
                    xg_t[:], x16_h[g * NPG:(g + 1) * NPG, :].rearrange(
                        "(j p) k -> p j k", p=128))
                evT_t = sp.tile([128, NPG], dt.float16, tag="evT", name="evT_t")
                nc.sync.dma_start(evT_t[:], evT_h[g * K:(g + 1) * K, :])
                dec_t = sp.tile([128, C], dt.float32, tag="dec", name="dec_t")
                nc.sync.dma_start(dec_t[:], dec_h[g])
                bc0 = int(cum_CB[g * BPG])
                bc1 = int(cum_CB[(g + 1) * BPG])
                band_t = sp.tile([128, bc1 - bc0, BW], dt.float8e4, tag="band",
                                 name="band_t")
                nc.sync.dma_start(band_t[:], band_h[:, bc0:bc1, :])
                return dict(em_t=em_t, xg_t=xg_t, evT_t=evT_t, dec_t=dec_t,
                            band_t=band_t, bc0=bc0)

            cur = load_graph(0)
            cur_next = [None]
            for g in range(GPC):
                em_t, xg_t = cur["em_t"], cur["xg_t"]
                evT_t, dec_t = cur["evT_t"], cur["dec_t"]
                band_t, bc0 = cur["band_t"], cur["bc0"]

                pxs = pp.tile([128, C], dt.float32, tag="p256", bufs=2,
                              name="pxs")
                for j in range(16):
                    nc.tensor.matmul(pxs[:], em_t[:, j, :], xg_t[:, j, :],
                                     start=(j == 0), stop=(j == 15))
                xsd = sp.tile([128, C], dt.float16, tag="xsd", name="xsd")
                nc.vector.tensor_mul(xsd[:], pxs[:], dec_t[:])

                diffT = [sp.tile([128, NPG], dt.float16, tag=f"diffT{h2}",
                                 name=f"diffT{h2}")
                         for h2 in range(2)]
                for h2 in range(2):
                    for j in range(4):
                        pd = pp.tile([128, 512], dt.float32, tag="w512", bufs=2,
                                     name="pd")
                        nc.tensor.matmul(pd[:],
                                         xsd[:, h2 * 128:(h2 + 1) * 128],
                                         evT_t[:, j * 512:(j + 1) * 512],
                                         start=True, stop=True)
                        nc.scalar.copy(diffT[h2][:, j * 512:(j + 1) * 512], pd[:])

                for b in range(BPG):
                    B = g * BPG + b
                    if b % 4 == 0:
                        segT_t = sg.tile([128, 2, 512], dt.float16, tag="segT",
                                         name="segT_t")
                    seg_block(B, band_t, bc0, segT_t)
                    if b == 1 and g + 1 < GPC:
                        cur_next[0] = load_graph(g + 1)
                    if b % 4 == 3:
                        mlp_group(B, segT_t, diffT, xg_t)
                cur = cur_next[0]
    _legalize_waits(nc)
    return nc


# ---------------------------------------------------------------- entry
def kernel(**inputs):
    in_maps, plan = _prepare(inputs)
    nc = _build(plan)
    res = bass_utils.run_bass_kernel_spmd(nc, in_maps,
                                          core_ids=list(range(NCORES)))
    return np.concatenate([res.results[i]["out"] for i in range(NCORES)],
                          0).astype(np.float32)
